# revision 47
# baseline (speedup 1.0000x reference)
"""GQA attention kernel for Trainium2, 8-core SPMD.

Sharding: core c = 2*b + g handles batch b (of 4) and head-group g (of 2):
8 of 16 q-heads, 2 of 4 kv-heads.  Each core computes its partial
out^T = (attn_out @ wo_g^T)^T in transposed space (no on-chip transposes);
the host adds the two group partials per batch and transposes back.

The kernel is Tensor-exec bound (~N/2.4 ns per matmul of moving size N,
any dtype), so everything that is not a real GEMM is kept OFF the PE:
  - causal mask: gpsimd.affine_select on the exp tiles (off the PE).
  - softmax denominator: DVE adds of exp tiles + ONE all-ones [128,128]
    matmul per q-tile that partition-reduces AND broadcasts in one shot,
    then reciprocal_approx_fast (DVE) and one DVE multiply to normalize.
  - RoPE pair-swap: SBUF->SBUF DMA partition swap (no PE perm-matmul).
  - fp16 storage everywhere; PSUM accumulation stays fp32.
  - x resident in SBUF (loaded once); Q and O never leave SBUF.
  - software pipeline: score matmuls run LAG pair-units ahead of the
    p-consuming matmuls in the in-order Tensor queue; Q-projection of
    head h+1 fills remaining PE gaps during attention of head h (spread
    evenly by quota); head 7 has no projection left, so out-projection
    tiles of already-normalized q-blocks fill its gaps instead.
  - phase 1 DMAs are split (wk/wv halves, x in half-T pieces) and issued
    in consumption order so the K/V matmuls chase the x transfer instead
    of stalling on full-tensor arrival.

Everything on-chip is computed in transposed orientation:
  Q^T/K^T: [head_dim(part), T]   scores^T: [kt(part), qt]   O^T: [d(part), qt]
RoPE is handled by permuting wq/wk rows on the host to an
[evens | odds] layout (scores are invariant to a shared d-permutation).
"""

import math
import numpy as np

B, T, C = 4, 2048, 2048
N_HEAD, N_KV_HEAD, HD = 16, 4, 128
N_CORES = 8
SCALE = 1.0 / math.sqrt(HD)

_PROG = {}
_LAST_IN_MAPS = None


def _build_program():
    from contextlib import ExitStack
    import concourse.bacc as bacc
    import concourse.mybir as mybir
    import concourse.tile as tile

    f16 = mybir.dt.float16
    f32 = mybir.dt.float32
    Exp = mybir.ActivationFunctionType.Exp

    nc = bacc.Bacc(None, target_bir_lowering=False)
    xH = nc.declare_dram_parameter("xH", [128, 16, T], f16, isOutput=False)
    wqH = nc.declare_dram_parameter("wqH", [8, 128, 16, 128], f16, isOutput=False)
    # wk/wv interleaved per contraction chunk (slot 2ci = wk_ci, 2ci+1 =
    # wv_ci) so one boot DMA carries the first chunks of both
    wkvH = nc.declare_dram_parameter("wkvH", [128, 32, 256], f16, isOutput=False)
    woH = nc.declare_dram_parameter("woH", [128, 8, T], f16, isOutput=False)
    cssnH = nc.declare_dram_parameter("cssn", [128, 2, T], f16, isOutput=False)
    out = nc.declare_dram_parameter("out", [C, T], f16, isOutput=True)

    with tile.TileContext(nc) as tc, nc.allow_low_precision(
        reason="fp16 storage with fp32 PSUM accumulation; tolerance is 2e-2"
    ), ExitStack() as top:
        consts = top.enter_context(tc.tile_pool(name="consts", bufs=1))
        cssn = consts.tile([128, 2, T], f16)
        cs2 = cssn[:, 0]
        sn2 = cssn[:, 1]
        ones_sq = consts.tile([128, 128], f16)
        nc.vector.memset(ones_sq, 1.0)

        data = top.enter_context(tc.tile_pool(name="data", bufs=1))
        x_sb = data.tile([128, 16, T], f16)
        K_sb = data.tile([128, 2, T], f16)
        V_sb = data.tile([128, 16, 256], f16)
        O_sb = data.tile([128, 8, T], f16)
        wo_sb = data.tile([128, 8, T], f16)

        # pools shared by K-rope (KV pass) and Q-proj/rope (attention era)
        qraws = top.enter_context(tc.tile_pool(name="qraws", bufs=3))
        ropes = top.enter_context(tc.tile_pool(name="ropes", bufs=2))
        ps_aux = top.enter_context(tc.tile_pool(name="ps_aux", bufs=1, space="PSUM"))
        wqp = top.enter_context(tc.tile_pool(name="wqp", bufs=2))
        qsbp = top.enter_context(tc.tile_pool(name="qsbp", bufs=2))
        pwork = top.enter_context(tc.tile_pool(name="pwork", bufs=3))
        dwork = top.enter_context(tc.tile_pool(name="dwork", bufs=1))
        dfold = top.enter_context(tc.tile_pool(name="dfold", bufs=1))
        rbcp = top.enter_context(tc.tile_pool(name="rbcp", bufs=1))
        # SBUF staging for out-projection tiles computed as head-7 filler
        obf = top.enter_context(tc.tile_pool(name="obf", bufs=2))

        def emit_rope(raw_f16, dst, tsl):
            # dst = raw*cs2 + swap_halves(raw)*sn2; the half-swap is a
            # partition-swapped SBUF->SBUF DMA copy (DMA engines are idle)
            sw = qraws.tile([128, 512], f16, tag="sw", name="sw")
            nc.sync.dma_start(out=sw[0:64, :], in_=raw_f16[64:128, :])
            nc.sync.dma_start(out=sw[64:128, :], in_=raw_f16[0:64, :])
            ta = ropes.tile([128, 512], f16, tag="ta", name="ta")
            tb = ropes.tile([128, 512], f16, tag="tb", name="tb")
            nc.vector.tensor_mul(ta, raw_f16, cs2[:, tsl])
            nc.vector.tensor_mul(tb, sw, sn2[:, tsl])
            nc.vector.tensor_add(dst, ta, tb)

        q_tiles = {}
        wq_tiles = {}

        def issue_wq(h):
            # issued ~a full head ahead of use: the [128,16,128] transfer is
            # ~128 descriptors on one queue and must not race its consumers
            wq_sb = wqp.tile([128, 16, 128], f16, tag="wq", name=f"wq{h}")
            wq_tiles[h] = wq_sb
            nc.sync.dma_start(out=wq_sb, in_=wqH[h])

        def qproj_ops(h):
            """Closure list computing Q_sb for head h (proj + rope)."""
            ops = []
            wq_sb = wq_tiles.pop(h)
            q_sb = qsbp.tile([128, T], f16, tag="q", name=f"q{h}")
            q_tiles[h] = q_sb
            for t4 in range(4):
                tsl = slice(t4 * 512, (t4 + 1) * 512)
                q_ps = ps_aux.tile([128, 512], f32, tag="aux", name="qp")
                for ci in range(16):
                    ops.append(lambda q_ps=q_ps, ci=ci, tsl=tsl: nc.tensor.matmul(
                        q_ps, wq_sb[:, ci, :], x_sb[:, ci, tsl],
                        start=(ci == 0), stop=(ci == 15),
                    ))
                def rope_q(q_ps=q_ps, tsl=tsl):
                    raw = qraws.tile([128, 512], f16, tag="raw", name="raw")
                    nc.scalar.copy(raw, q_ps)
                    emit_rope(raw, q_sb[:, tsl], tsl)
                ops.append(rope_q)
            return ops

        class ListFiller:
            """Pops ops from a fixed list, spread evenly over the era's
            filler points so the PE never runs dry mid-head."""

            def __init__(self, ops, n_points):
                self.ops = ops
                self.n_points = max(1, n_points)
                self.point = 0
                self.emitted = 0

            def at_point(self, boost=0):
                self.point += 1
                target = (len(self.ops) * self.point + self.n_points - 1) \
                    // self.n_points + boost
                while self.emitted < target and self.emitted < len(self.ops):
                    self.ops[self.emitted]()
                    self.emitted += 1

            def flush(self):
                while self.emitted < len(self.ops):
                    self.ops[self.emitted]()
                    self.emitted += 1

            def on_normalize(self, qj):
                pass

        # ---- out-projection emission (used as head-7 filler AND phase 3) --
        op_done = set()  # (e, tj) tiles already emitted as filler

        def outproj_tile_ops(e, tj, psum_pool, sb_pool, split):
            """Closure list: accumulate out tile (e, tj), copy, DMA.
            ops[7] is the only op reading head 7's O row."""
            tsl = slice(tj * 512, (tj + 1) * 512)
            ops = []
            state = {}

            def mm(hh):
                if hh == 0:
                    state["ps"] = psum_pool.tile([128, 512], f32, tag="aux",
                                                 name="opf")
                nc.tensor.matmul(
                    state["ps"], wo_sb[:, hh, e * 128:(e + 1) * 128],
                    O_sb[:, hh, tsl], start=(hh == 0), stop=(hh == 7),
                )
            for hh in range(8):
                ops.append(lambda hh=hh: mm(hh))

            def fin():
                ob = sb_pool.tile([128, 512], f16, tag="ob", name="ob")
                nc.vector.tensor_copy(ob, state["ps"])
                if split:
                    # split so the final transfer does not serialize 128
                    # descriptors on one queue
                    for sl in range(4):
                        nc.sync.dma_start(
                            out=out[e * 128 + 32 * sl:e * 128 + 32 * (sl + 1), tsl],
                            in_=ob[32 * sl:32 * (sl + 1)],
                        )
                else:
                    nc.sync.dma_start(out=out[e * 128:(e + 1) * 128, tsl], in_=ob)
            ops.append(fin)
            return ops

        class OutprojFiller:
            """Head-7 filler: emits out-projection tiles for q-blocks whose
            normalize has already been emitted (lagged one qj-group so the
            in-order PE queue never stalls on the DVE normalize chain)."""

            def __init__(self):
                from collections import deque
                self.tasks = deque()
                self.cur = None
                self.cur_key = None
                self.point = 0
                # prefill: heads 0-6 of tile (0,0) can accumulate during
                # head 7's earliest units, before any normalize has landed;
                # only ops[7:] (the hh=7 matmul + fin) must wait
                self.pre_key = (0, 0)
                self.pre_cur = outproj_tile_ops(0, 0, ps_aux, obf, split=False)

            def on_normalize(self, qj):
                if 0 <= qj <= 2:
                    # 3-point warmup so the first reader of O_sb[:,7,tj]
                    # enters the in-order PE queue well after the DVE
                    # normalize chain has drained
                    for e in range(16):
                        self.tasks.append((self.point + 3, e, qj))

            def _pop_one(self):
                if self.cur is None:
                    while (self.tasks and self.tasks[0][0] <= self.point
                           and (self.tasks[0][1], self.tasks[0][2]) in op_done):
                        self.tasks.popleft()
                    if self.tasks and self.tasks[0][0] <= self.point:
                        _, e, tj = self.tasks.popleft()
                        if (e, tj) == self.pre_key and self.pre_cur is not None:
                            self.cur = self.pre_cur  # finish the prefill
                            self.pre_cur = None
                        else:
                            self.cur = outproj_tile_ops(e, tj, ps_aux, obf,
                                                        split=False)
                        self.cur_key = (e, tj)
                    elif self.pre_cur is not None and len(self.pre_cur) > 2:
                        self.pre_cur.pop(0)()
                        return True
                    else:
                        return False
                self.cur.pop(0)()
                if not self.cur:
                    op_done.add(self.cur_key)
                    self.cur = None
                return True

            def at_point(self, boost=0):
                self.point += 1
                for _ in range(2 + boost):
                    if not self._pop_one():
                        return

            def flush(self):
                # finish the tiles in flight (the prefill's PSUM group must
                # be closed) plus a small budget of extra tiles: they fill
                # the PE while the last normalizes' DVE chain drains.
                while self.cur is not None:
                    self._pop_one()
                if self.pre_cur is not None:
                    while self.pre_cur:
                        self.pre_cur.pop(0)()
                    self.pre_cur = None
                    op_done.add(self.pre_key)
                self.point += 10**6
                for _ in range(2 * 9):
                    if not self._pop_one():
                        break

        # ---- phase 1: K/V projections + K RoPE (x resident in SBUF) ----
        with ExitStack() as kv_stack:
            wkv = kv_stack.enter_context(tc.tile_pool(name="wkv", bufs=1))
            wkv_sb = wkv.tile([128, 32, 256], f16)
            # DMA order matters (per-queue FIFO): everything is issued in
            # consumption order so the first K/V matmuls start as soon as
            # the first x pieces land instead of waiting for full tensors.
            # Boot transfers are coalesced (the ~650ns per-DMA trigger cost
            # on the sync queue serializes early arrivals otherwise).
            nc.sync.dma_start(out=wkv_sb[:, 0:4], in_=wkvH[:, 0:4])
            nc.sync.dma_start(out=x_sb[:, 0, 0:1024], in_=xH[:, 0, 0:1024])
            nc.sync.dma_start(out=wkv_sb[:, 4:8], in_=wkvH[:, 4:8])
            nc.sync.dma_start(out=x_sb[:, 1, 0:1024], in_=xH[:, 1, 0:1024])
            nc.sync.dma_start(out=wkv_sb[:, 8:16], in_=wkvH[:, 8:16])
            for ci in range(2, 5):
                nc.sync.dma_start(out=x_sb[:, ci, 0:1024], in_=xH[:, ci, 0:1024])
            nc.sync.dma_start(out=wkv_sb[:, 16:24], in_=wkvH[:, 16:24])
            for ci in range(5, 7):
                nc.sync.dma_start(out=x_sb[:, ci, 0:1024], in_=xH[:, ci, 0:1024])
            nc.sync.dma_start(out=wkv_sb[:, 24:32], in_=wkvH[:, 24:32])
            for ci in range(7, 16):
                nc.sync.dma_start(out=x_sb[:, ci, 0:1024], in_=xH[:, ci, 0:1024])
            nc.sync.dma_start(out=cssn[:, :, 0:1024], in_=cssnH[:, :, 0:1024])
            # head 0/1 q-weights: needed only by the q0 filler in the late
            # KV blocks, so they queue behind the critical h0 x pieces
            issue_wq(0)
            issue_wq(1)
            for c4 in range(4):
                nc.sync.dma_start(out=x_sb[:, 4 * c4:4 * (c4 + 1), 1024:2048],
                                  in_=xH[:, 4 * c4:4 * (c4 + 1), 1024:2048])
            nc.sync.dma_start(out=cssn[:, :, 1024:2048], in_=cssnH[:, :, 1024:2048])
            # head 0's projection is interleaved into the KV pass (after
            # x has mostly arrived, so it never blocks)
            ops0 = qproj_ops(0)
            ps_k = kv_stack.enter_context(tc.tile_pool(name="ps_k", bufs=4, space="PSUM"))
            ps_v = kv_stack.enter_context(tc.tile_pool(name="ps_v", bufs=2, space="PSUM"))

            def emit_v(t4, ci, v_ps, s2):
                for sub in (s2, s2 + 1):
                    nc.tensor.matmul(
                        v_ps[sub - s2],
                        x_sb[:, ci, t4 * 512 + sub * 128:t4 * 512 + (sub + 1) * 128],
                        wkv_sb[:, 2 * ci + 1, :], start=(ci == 0), stop=(ci == 15),
                    )

            def fin_v(t4, v_ps, s2):
                for sub in (s2, s2 + 1):
                    nc.scalar.copy(V_sb[:, t4 * 4 + sub, :], v_ps[sub - s2])

            def v_half(t4, s2, pt=None):
                v_ps = [ps_v.tile([128, 256], f32, tag="vps", name=f"vps{i}")
                        for i in range(2)]
                for ci in range(16):
                    emit_v(t4, ci, v_ps, s2)
                    if pt is not None:
                        pt()
                fin_v(t4, v_ps, s2)

            def fin_k(t4, k_ps):
                tsl = slice(t4 * 512, (t4 + 1) * 512)
                for kv in range(2):
                    raw = qraws.tile([128, 512], f16, tag="raw", name="raw")
                    nc.scalar.copy(raw, k_ps[kv])
                    emit_rope(raw, K_sb[:, kv, tsl], tsl)

            # pass A: K for t4=0,1 plus half of V t4=0, ci-major, so the
            # per-ci PE work (~1.1us) keeps pace with the x piece arrivals
            # and the PE never stalls waiting for the tail of the x stream
            k_ps01 = [[ps_k.tile([128, 512], f32, tag="kps", name=f"kps{t}{i}")
                       for i in range(2)] for t in range(2)]
            v_ps0 = [ps_v.tile([128, 256], f32, tag="vps", name=f"vps{i}")
                     for i in range(2)]
            for ci in range(16):
                for t4 in range(2):
                    for kv in range(2):
                        nc.tensor.matmul(
                            k_ps01[t4][kv], wkv_sb[:, 2 * ci, kv * 128:(kv + 1) * 128],
                            x_sb[:, ci, t4 * 512:(t4 + 1) * 512],
                            start=(ci == 0), stop=(ci == 15),
                        )
                emit_v(0, ci, v_ps0, 0)
            fin_v(0, v_ps0, 0)
            fin_k(0, k_ps01[0])
            fin_k(1, k_ps01[1])
            # q0 projection ops are spread by quota over all remaining
            # phase-1 points: the 512-wide q matmuls break the chains of
            # 256-wide V matmuls whose weight loads are otherwise exposed
            p1f = {"done": 0, "pt": 0}
            N_PTS = 160

            def pt():
                p1f["pt"] += 1
                target = (len(ops0) * p1f["pt"] + N_PTS - 1) // N_PTS
                while p1f["done"] < min(target, len(ops0)):
                    ops0[p1f["done"]]()
                    p1f["done"] += 1

            # remaining V halves (x resident by now)
            v_half(0, 2, pt)
            v_half(1, 0, pt)
            v_half(1, 2, pt)
            for t4 in range(2, 4):
                k_ps = [ps_k.tile([128, 512], f32, tag="kps", name=f"kps{i}")
                        for i in range(2)]
                for ci in range(16):
                    for kv in range(2):
                        nc.tensor.matmul(
                            k_ps[kv], wkv_sb[:, 2 * ci, kv * 128:(kv + 1) * 128],
                            x_sb[:, ci, t4 * 512:(t4 + 1) * 512],
                            start=(ci == 0), stop=(ci == 15),
                        )
                    pt()
                fin_k(t4, k_ps)
                v_half(t4, 0, pt)
                v_half(t4, 2, pt)
            while p1f["done"] < len(ops0):
                ops0[p1f["done"]]()
                p1f["done"] += 1

        # ---- attention era: software-pipelined per head ----
        # Score matmuls run LAG pair-units ahead of the p-consuming matmuls
        # in the (in-order) Tensor queue, so the scalar engine's exp stream
        # runs back-to-back.  Scores for two adjacent 128-wide k-blocks
        # share one [128,1024] PSUM tile -> one wide exp.  Causal masking
        # happens on the exp tiles via gpsimd affine_select (off the PE).
        with ExitStack() as at_stack:
            ps_s = at_stack.enter_context(tc.tile_pool(name="ps_s", bufs=2, space="PSUM"))
            ps_o = at_stack.enter_context(tc.tile_pool(name="ps_o", bufs=2, space="PSUM"))
            ps_den = at_stack.enter_context(tc.tile_pool(name="ps_den", bufs=1, space="PSUM"))
            LAG = 2

            def emit_attn(h, filler):
                kv = h // 4
                q_sb = q_tiles.pop(h)
                units = []
                for qj in range(4):
                    nk = 4 * (qj + 1)
                    for kp in range(nk // 2):
                        units.append((qj, 2 * kp, 2 * kp + 1))
                n = len(units)
                p_tiles = [None] * n
                qj_state = {}

                def emit_score(i):
                    qj, k0, k1 = units[i]
                    s2 = ps_s.tile([128, 1024], f32, tag="s", name="s2")
                    for j, ki in enumerate((k0, k1)):
                        # diagonal blocks (r>0): fully-masked columns qt <
                        # r*128 are never computed; exp of the stale PSUM
                        # there is finite garbage that affine_select zeroes.
                        r = max(ki - 4 * qj, 0)
                        nc.tensor.matmul(
                            s2[:, j * 512 + r * 128:(j + 1) * 512],
                            K_sb[:, kv, ki * 128:(ki + 1) * 128],
                            q_sb[:, qj * 512 + r * 128:(qj + 1) * 512],
                            start=True, stop=True,
                        )
                    p2 = pwork.tile([128, 1024], f16, tag="p", name="p2")
                    nc.scalar.activation(p2, s2, Exp, scale=SCALE)
                    for j, ki in enumerate((k0, k1)):
                        if ki >= 4 * qj:  # diagonal block: zero masked region
                            nc.gpsimd.affine_select(
                                out=p2[:, j * 512:(j + 1) * 512],
                                in_=p2[:, j * 512:(j + 1) * 512],
                                pattern=[[1, 512]],
                                compare_op=mybir.AluOpType.is_ge, fill=0.0,
                                base=qj * 512 - ki * 128, channel_multiplier=-1,
                            )
                    p_tiles[i] = p2

                pending = []

                def emit_normalize():
                    # deferred by one unit so the den-reduce matmul never
                    # head-of-line blocks the Tensor queue on the DVE fold
                    o_ps, den_f, qsl, qj = pending.pop(0)
                    den_ps = ps_den.tile([128, 512], f32, tag="dn", name="den_ps")
                    nc.tensor.matmul(den_ps, ones_sq, den_f)
                    rbc = rbcp.tile([128, 512], f32, tag="rbc", name="rbc")
                    nc.vector.reciprocal_approx_fast(out=rbc, in_=den_ps)
                    nc.vector.tensor_mul(O_sb[:, h, qsl], o_ps, rbc)
                    filler.on_normalize(qj)

                def emit_consume(i):
                    qj, k0, k1 = units[i]
                    qsl = slice(qj * 512, (qj + 1) * 512)
                    nk = 4 * (qj + 1)
                    if k0 == 0:
                        qj_state[qj] = (
                            ps_o.tile([128, 512], f32, tag="o", name="o_ps"),
                            dwork.tile([128, 1024], f16, tag="dw", name="den_w"),
                        )
                    o_ps, den_w = qj_state[qj]
                    p2 = p_tiles[i]
                    p_tiles[i] = None
                    if k0 == 0:
                        nc.vector.tensor_copy(den_w, p2)
                    else:
                        nc.vector.tensor_add(den_w, den_w, p2)
                    for j, ki in enumerate((k0, k1)):
                        r = max(ki - 4 * qj, 0)
                        nc.tensor.matmul(
                            o_ps[:, r * 128:512],
                            V_sb[:, ki, kv * 128:(kv + 1) * 128],
                            p2[:, j * 512 + r * 128:(j + 1) * 512],
                            start=(ki == 0), stop=(ki == nk - 1),
                        )
                    if pending:
                        emit_normalize()
                    if k1 == nk - 1:
                        den_f = dfold.tile([128, 512], f16, tag="df", name="den_f")
                        nc.vector.tensor_add(den_f, den_w[:, 0:512], den_w[:, 512:1024])
                        pending.append((o_ps, den_f, qsl, qj))

                for i in range(n + LAG):
                    if i < n:
                        emit_score(i)
                        cqj, ck0, ck1 = units[i - LAG] if i >= LAG else (9, 0, 0)
                        filler.at_point(boost=2 if ck1 >= 4 * cqj else 0)
                    if i >= LAG:
                        emit_consume(i - LAG)
                        filler.at_point()
                while pending:
                    emit_normalize()
                filler.flush()

            nc.sync.dma_start(out=wo_sb, in_=woH[:])
            for h in range(8):
                if h + 2 <= 7:
                    issue_wq(h + 2)
                if h < 7:
                    filler = ListFiller(qproj_ops(h + 1), n_points=2 * 20)
                else:
                    filler = OutprojFiller()
                emit_attn(h, filler)

        # ---- phase 3: output projection (transposed partials) ----
        # Remaining (e, tj) tiles; adjacent tj runs of an e share one wide
        # SBUF staging tile and one DMA (full-T rows = 4 KB descriptors,
        # ~4x the drain rate of per-tj 1 KB rows).
        with ExitStack() as ph3:
            outsb = ph3.enter_context(tc.tile_pool(name="outsb", bufs=2))
            ps_out = ph3.enter_context(tc.tile_pool(name="ps_out", bufs=6, space="PSUM"))
            runs = []
            for e in range(16):
                tjs = [tj for tj in range(4) if (e, tj) not in op_done]
                start = None
                for tj in range(5):
                    if tj < 4 and tj in tjs:
                        if start is None:
                            start = tj
                    elif start is not None:
                        runs.append((e, start, tj))
                        start = None
            if runs and runs[-1][2] - runs[-1][1] > 1:
                # peel the final strip off the last run so the post-last-
                # matmul drain is one 512-col strip, not a whole run
                e, tj0, tj1 = runs.pop()
                runs.append((e, tj0, tj1 - 1))
                runs.append((e, tj1 - 1, tj1))
            for ri, (e, tj0, tj1) in enumerate(runs):
                tail = ri >= len(runs) - 2
                ob = outsb.tile([128, 2048], f16, tag="ob", name="ob")
                obv = ob[:, tj0 * 512:tj1 * 512]
                esl = slice(e * 128, (e + 1) * 128)
                for tj in range(tj0, tj1):
                    op_ = ps_out.tile([128, 512], f32, tag="op", name="op")
                    for hh in range(8):
                        nc.tensor.matmul(
                            op_, wo_sb[:, hh, e * 128:(e + 1) * 128],
                            O_sb[:, hh, tj * 512:(tj + 1) * 512],
                            start=(hh == 0), stop=(hh == 7),
                        )
                    nc.vector.tensor_copy(
                        ob[:, tj * 512:(tj + 1) * 512], op_)
                    if tail:
                        # final runs: fire each 512-col strip right after
                        # its copy (scalar HWDGE queue, idle here) so only
                        # the last strip drains after the last matmul
                        nc.scalar.dma_start(
                            out=out[esl, tj * 512:(tj + 1) * 512],
                            in_=ob[:, tj * 512:(tj + 1) * 512],
                        )
                if not tail:
                    nc.scalar.dma_start(
                        out=out[esl, tj0 * 512:tj1 * 512], in_=obv)

    nc.compile()
    return nc


def _get_program():
    if "nc" not in _PROG:
        _PROG["nc"] = _build_program()
    return _PROG["nc"]


def kernel(x, wq, wk, wv, wo, rope_cos, rope_sin):
    from concourse.bass_utils import run_bass_kernel_spmd

    nc = _get_program()
    x = np.asarray(x, dtype=np.float32)
    wq = np.asarray(wq, dtype=np.float32)
    wk = np.asarray(wk, dtype=np.float32)
    wv = np.asarray(wv, dtype=np.float32)
    wo = np.asarray(wo, dtype=np.float32)
    cosT = np.asarray(rope_cos, dtype=np.float32).T  # [64, T]
    sinT = np.asarray(rope_sin, dtype=np.float32).T

    # even/odd -> [evens | odds] permutation of each head's rows of wq/wk
    perm = np.concatenate([np.arange(0, HD, 2), np.arange(1, HD, 2)])
    wq_p = wq.reshape(N_HEAD, HD, C)[:, perm, :]          # [16, 128, C]
    wk_p = wk.reshape(N_KV_HEAD, HD, C)[:, perm, :]       # [4, 128, C]
    wv_r = wv.reshape(N_KV_HEAD, HD, C)                   # [4, 128, C]

    cos2 = np.concatenate([cosT, cosT], axis=0).astype(np.float16)
    sin2 = np.concatenate([-sinT, sinT], axis=0).astype(np.float16)
    cssn = np.ascontiguousarray(np.stack([cos2, sin2], axis=1))  # [128, 2, T]

    def part_major(a):  # [rows(c=n*128+p), m] -> [128(p), n, m]
        rows, m = a.shape
        return np.ascontiguousarray(
            a.reshape(rows // 128, 128, m).transpose(1, 0, 2))

    in_maps = []
    for core in range(N_CORES):
        b, g = core // 2, core % 2
        xT = x[b].T.astype(np.float16)                     # [C, T]
        wq_g = wq_p[8 * g:8 * g + 8]                       # [8, 128, C]
        wqHa = np.stack([part_major(wq_g[hl].T.astype(np.float16))
                         for hl in range(8)])              # [8, 128, 16, 128]
        wkHa = np.concatenate(
            [part_major(wk_p[2 * g + kv].T.astype(np.float16))
             for kv in range(2)], axis=2)                  # [128, 16, 256]
        wvHa = np.concatenate(
            [part_major(wv_r[2 * g + kv].T.astype(np.float16))
             for kv in range(2)], axis=2)
        # interleave: slot 2ci = wk chunk ci, slot 2ci+1 = wv chunk ci
        wkvHa = np.empty((128, 32, 256), dtype=np.float16)
        wkvHa[:, 0::2] = wkHa
        wkvHa[:, 1::2] = wvHa
        wo_g = wo[:, 1024 * g:1024 * (g + 1)]              # [C(e), 1024(hd)]
        woHa = part_major(wo_g.T.astype(np.float16))
        # woHa: rows = hd = hl*128 + p -> [128(p), 8(hl), C(e)]
        in_maps.append({
            "xH": part_major(xT),
            "wqH": wqHa,
            "wkvH": wkvHa,
            "woH": woHa,
            "cssn": cssn,
        })

    global _LAST_IN_MAPS
    _LAST_IN_MAPS = in_maps
    res = run_bass_kernel_spmd(nc, in_maps, list(range(N_CORES))).results
    outp = np.empty((B, T, C), dtype=np.float32)
    for b in range(B):
        outp[b] = (res[2 * b]["out"].astype(np.float32)
                   + res[2 * b + 1]["out"].astype(np.float32)).T
    return outp
